# revision 6
# baseline (speedup 1.0000x reference)
"""nn_AdaptiveGraphLayer Trainium2 kernel (8 NeuronCores, SPMD).

Sharding: each core owns N/8 = 512 query rows for all H=4 heads.
 - x (node features) replicated -> every core computes K/V for all nodes.
 - adjacency mask built on host, sharded by query rows (additive, [512,4096])
   and by key rows transposed (multiplicative, [4096,512]).
 - outputs (attn shard [4,512,4096], out shard [512,256]) are gathered on host,
   so no device collectives are needed.

Device pipeline per core:
  proj:  QT/KT (bf16, head-dim on partitions) and V (natural layout) via PE.
  A:     scores = Q^T.T @ K^T per (head, q-chunk); DVE adds additive mask while
         evacuating PSUM; ACT exp with fused row-sum accum; normalize on DVE;
         DMA attn shard out (bf16).
  B:     scores^T per (head, k-chunk); ACT exp from PSUM; mask-multiply (DVE /
         GPSIMD); PE-accumulate U_h = V_h^T @ expT into PSUM.
  C:     PE-transpose U_h, scale by 1/rowsum.
  D:     residual + LayerNorm (bn_stats/bn_aggr), affine, DMA out rows.
"""

import numpy as np

B, N, F, H, HD = 1, 4096, 256, 4, 64
NCORES = 8
NQ = N // NCORES          # 512 query rows per core
P = 128
NEG = -30000.0            # additive mask value; exp() underflows to exactly 0
LN_EPS = 1e-5
GPS_KC_MOD = 4            # kc % GPS_KC_MOD == 0 -> mask-mult on GPSIMD (B phase)

TRACE = False             # set True (with ntff shim installed) to profile
LAST_EXEC_NS = None
LAST_RESULTS = None

_BUILT = None


def _build():
    from contextlib import ExitStack

    import concourse.bacc as bacc
    import concourse.mybir as mybir
    from concourse.tile import TileContext
    from concourse.masks import make_identity

    dt = mybir.dt
    f32, bf16 = dt.float32, dt.bfloat16
    AF = mybir.ActivationFunctionType
    OP = mybir.AluOpType

    nc = bacc.Bacc("TRN2", target_bir_lowering=False, debug=False,
                   num_devices=NCORES)

    def din(name, shape, dtype):
        return nc.declare_dram_parameter(name, list(shape), dtype, isOutput=False)

    def dout(name, shape, dtype):
        return nc.declare_dram_parameter(name, list(shape), dtype, isOutput=True)

    xT = din("xT", [2, P, N], bf16)          # x^T [f_in, node], f_in-chunked
    xqT = din("xqT", [2, P, NQ], bf16)       # this core's q columns of x^T
    xr = din("xr", [NQ, F], f32)             # residual rows (f32)
    wqT = din("wqT", [2, P, F], bf16)        # Wq.T [f_in, out], f_in-chunked
    wkT = din("wkT", [2, P, F], bf16)
    wvT = din("wvT", [2, P, F], bf16)
    bq8 = din("bq8", [2, P, 1], f32)         # bq / sqrt(hd)
    bkb = din("bkb", [2, P, 1], f32)
    bvb = din("bvb", [1, F], f32)
    lng = din("lng", [1, F], f32)
    lnb = din("lnb", [1, F], f32)
    madd = din("madd", [4, P, N], bf16)      # additive mask rows, qc-chunked
    mmulT = din("mmulT", [32, P, NQ], bf16)  # multiplicative mask^T, kc-chunked
    attn_d = dout("attn", [H, NQ, N], bf16)
    out_d = dout("outp", [NQ, F], f32)

    with TileContext(nc) as tc, ExitStack() as ctx:
        const = ctx.enter_context(tc.tile_pool(name="const", bufs=1))
        big = ctx.enter_context(tc.tile_pool(name="big", bufs=1))
        wk_sm = ctx.enter_context(tc.tile_pool(name="wk_sm", bufs=2))
        wk_ex = ctx.enter_context(tc.tile_pool(name="wk_ex", bufs=2))
        wk_at = ctx.enter_context(tc.tile_pool(name="wk_at", bufs=2))
        wk_b = ctx.enter_context(tc.tile_pool(name="wk_b", bufs=2))
        wk_d = ctx.enter_context(tc.tile_pool(name="wk_d", bufs=1))
        pmm = ctx.enter_context(tc.tile_pool(name="pmm", bufs=2, space="PSUM"))
        pu = ctx.enter_context(tc.tile_pool(name="pu", bufs=2, space="PSUM"))
        pt = ctx.enter_context(tc.tile_pool(name="pt", bufs=2, space="PSUM"))

        # ---- constants / parameters to SBUF ----
        ident = const.tile([HD, HD], f32)
        make_identity(nc, ident[:])

        xT_sb = big.tile([P, 2, N], bf16)
        xqT_sb = big.tile([P, 2, NQ], bf16)
        KT_sb = big.tile([P, 2, N], bf16)
        QT_sb = big.tile([P, 2, NQ], bf16)
        V_sb = big.tile([P, 32, F], bf16)
        madd_sb = big.tile([P, 4, N], bf16)
        mmulT_sb = big.tile([P, 32, NQ], bf16)
        w_sb = {}
        for nm, t in (("q", wqT), ("k", wkT), ("v", wvT)):
            w_sb[nm] = big.tile([P, 2, F], bf16, tag=f"w{nm}", name=f"w{nm}_sb")
            for kc in range(2):
                nc.sync.dma_start(out=w_sb[nm][:, kc, :], in_=t[kc])
        for kc in range(2):
            nc.sync.dma_start(out=xT_sb[:, kc, :], in_=xT[kc])
            nc.sync.dma_start(out=xqT_sb[:, kc, :], in_=xqT[kc])
        for qc in range(4):
            nc.sync.dma_start(out=madd_sb[:, qc, :], in_=madd[qc])
        for kc in range(32):
            nc.sync.dma_start(out=mmulT_sb[:, kc, :], in_=mmulT[kc])

        bq8_sb = const.tile([P, 2], f32)
        bk_sb = const.tile([P, 2], f32)
        for mc in range(2):
            nc.sync.dma_start(out=bq8_sb[:, mc:mc + 1], in_=bq8[mc])
            nc.sync.dma_start(out=bk_sb[:, mc:mc + 1], in_=bkb[mc])

        import concourse.bass as bass

        def bcast(dram_ap):
            # replicate a [1, F] dram row across 128 partitions
            return bass.AP(tensor=dram_ap.tensor, offset=dram_ap.offset,
                           ap=[[0, P]] + list(dram_ap.ap[1:]))

        bv_bc = const.tile([P, F], f32)
        g_bc = const.tile([P, F], f32)
        b_bc = const.tile([P, F], f32)
        nc.sync.dma_start(out=bv_bc[:], in_=bcast(bvb[:]))
        nc.sync.dma_start(out=g_bc[:], in_=bcast(lng[:]))
        nc.sync.dma_start(out=b_bc[:], in_=bcast(lnb[:]))

        xr_sb = const.tile([P, 4, F], f32)
        for qc in range(4):
            nc.sync.dma_start(out=xr_sb[:, qc, :], in_=xr[qc * P:(qc + 1) * P, :])

        eps_sb = const.tile([P, 1], f32)
        nc.vector.memset(eps_sb[:], LN_EPS)
        sums_sb = const.tile([P, H * 4], f32)
        rinv_sb = const.tile([P, H * 4], f32)
        out_sb = const.tile([P, 4, F], f32)

        # ---- projections ----
        # QT [256,512]: out chunk mc on partitions
        for mc in range(2):
            ps = pmm.tile([P, 1024], f32, tag="mm")
            for kc in range(2):
                nc.tensor.matmul(ps[:, 0:NQ],
                                 w_sb["q"][:, kc, mc * P:(mc + 1) * P],
                                 xqT_sb[:, kc, :],
                                 start=(kc == 0), stop=(kc == 1))
            nc.scalar.activation(out=QT_sb[:, mc, :], in_=ps[:, 0:NQ],
                                 func=AF.Identity, bias=bq8_sb[:, mc:mc + 1],
                                 scale=1.0 / np.sqrt(HD))
        # KT [256,4096]
        for mc in range(2):
            for n2 in range(4):      # 4 chunks of 1024
                ps = pmm.tile([P, 1024], f32, tag="mm")
                for j in range(2):
                    n0 = n2 * 1024 + j * 512
                    for kc in range(2):
                        nc.tensor.matmul(ps[:, j * 512:(j + 1) * 512],
                                         w_sb["k"][:, kc, mc * P:(mc + 1) * P],
                                         xT_sb[:, kc, n0:n0 + 512],
                                         start=(kc == 0), stop=(kc == 1))
                nc.scalar.activation(out=KT_sb[:, mc, n2 * 1024:(n2 + 1) * 1024],
                                     in_=ps[:], func=AF.Identity,
                                     bias=bk_sb[:, mc:mc + 1], scale=1.0)
        # V natural [node, 256]
        for nc32 in range(32):
            ps = pmm.tile([P, 1024], f32, tag="mm")
            for kc in range(2):
                nc.tensor.matmul(ps[:, 0:F],
                                 xT_sb[:, kc, nc32 * P:(nc32 + 1) * P],
                                 w_sb["v"][:, kc, :],
                                 start=(kc == 0), stop=(kc == 1))
            nc.vector.scalar_tensor_tensor(out=V_sb[:, nc32, :], in0=ps[:, 0:F],
                                           scalar=0.0, in1=bv_bc[:],
                                           op0=OP.bypass, op1=OP.add)

        def qk_slice(t, h, lo, sz):
            # head h slice of a [P, 2, *] head-major tensor: partitions
            # (h%2)*64..+64, chunk h//2, free lo..lo+sz
            return t[(h % 2) * HD:(h % 2 + 1) * HD, h // 2, lo:lo + sz]

        # ---- phase A: qk scores -> masked exp -> rowsums -> attn out ----
        for h in range(H):
            for qc in range(4):
                idx = h * 4 + qc
                sm = wk_sm.tile([P, N], bf16, tag="sm")
                for t4 in range(4):          # 4 psum tiles of 1024 keys
                    ps = pmm.tile([P, 1024], f32, tag="mm")
                    for j in range(2):
                        k0 = t4 * 1024 + j * 512
                        nc.tensor.matmul(ps[:, j * 512:(j + 1) * 512],
                                         qk_slice(QT_sb, h, qc * P, P),
                                         qk_slice(KT_sb, h, k0, 512),
                                         start=True, stop=True)
                    nc.vector.scalar_tensor_tensor(
                        out=sm[:, t4 * 1024:(t4 + 1) * 1024], in0=ps[:],
                        scalar=0.0, in1=madd_sb[:, qc, t4 * 1024:(t4 + 1) * 1024],
                        op0=OP.bypass, op1=OP.add)
                ex = wk_ex.tile([P, N], bf16, tag="ex")
                nc.scalar.activation(out=ex[:], in_=sm[:], func=AF.Exp,
                                     accum_out=sums_sb[:, idx:idx + 1])
                nc.vector.reciprocal(out=rinv_sb[:, idx:idx + 1],
                                     in_=sums_sb[:, idx:idx + 1])
                at = wk_at.tile([P, N], bf16, tag="at")
                nc.vector.tensor_scalar_mul(at[:], ex[:],
                                            rinv_sb[:, idx:idx + 1])
                nc.sync.dma_start(out=attn_d[h, qc * P:(qc + 1) * P, :],
                                  in_=at[:])

        # ---- phase B: transposed scores -> exp -> mask -> U accumulation ----
        u_psums = {}
        for pair in range(2):
            h0, h1 = 2 * pair, 2 * pair + 1
            u0 = pu.tile([HD, NQ], f32, tag="u")
            u1 = pu.tile([HD, NQ], f32, tag="u")
            u_psums[h0], u_psums[h1] = u0, u1
            for kc in range(32):
                ps = pmm.tile([P, 1024], f32, tag="mm")
                for j, h in enumerate((h0, h1)):
                    nc.tensor.matmul(ps[:, j * NQ:(j + 1) * NQ],
                                     qk_slice(KT_sb, h, kc * P, P),
                                     qk_slice(QT_sb, h, 0, NQ),
                                     start=True, stop=True)
                et = wk_b.tile([P, 1024], bf16, tag="et")
                nc.scalar.activation(out=et[:], in_=ps[:], func=AF.Exp)
                em = wk_b.tile([P, 1024], bf16, tag="em")
                eng = nc.gpsimd if (kc % GPS_KC_MOD == 0) else nc.vector
                for j in range(2):
                    eng.tensor_tensor(out=em[:, j * NQ:(j + 1) * NQ],
                                      in0=et[:, j * NQ:(j + 1) * NQ],
                                      in1=mmulT_sb[:, kc, :], op=OP.mult)
                for j, h in enumerate((h0, h1)):
                    nc.tensor.matmul(u_psums[h],
                                     V_sb[:, kc, h * HD:(h + 1) * HD],
                                     em[:, j * NQ:(j + 1) * NQ],
                                     start=(kc == 0), stop=(kc == 31))
            # ---- phase C for this pair: evac, transpose, scale ----
            for h in (h0, h1):
                us = wk_b.tile([HD, NQ], f32, tag="us")
                nc.scalar.activation(out=us[:], in_=u_psums[h], func=AF.Copy)
                for qc in range(4):
                    tp = pt.tile([P, HD], f32, tag="tp")
                    nc.tensor.transpose(tp[:], us[:, qc * P:(qc + 1) * P],
                                        ident[:])
                    nc.vector.tensor_scalar_mul(
                        out_sb[:, qc, h * HD:(h + 1) * HD], tp[:],
                        rinv_sb[:, h * 4 + qc:h * 4 + qc + 1])

        # ---- phase D: residual + layernorm + affine ----
        for qc in range(4):
            y = wk_d.tile([P, F], f32, tag="y")
            nc.vector.tensor_tensor(out=y[:], in0=out_sb[:, qc, :],
                                    in1=xr_sb[:, qc, :], op=OP.add)
            st = wk_d.tile([P, 6], f32, tag="st")
            nc.vector.bn_stats(out=st[:], in_=y[:])
            mv = wk_d.tile([P, 2], f32, tag="mv")
            nc.vector.bn_aggr(out=mv[:], in_=st[:])
            sd = wk_d.tile([P, 1], f32, tag="sd")
            nc.scalar.activation(out=sd[:], in_=mv[:, 1:2], func=AF.Sqrt,
                                 bias=eps_sb[:], scale=1.0)
            rs = wk_d.tile([P, 1], f32, tag="rs")
            nc.vector.reciprocal(out=rs[:], in_=sd[:])
            yc = wk_d.tile([P, F], f32, tag="yc")
            nc.vector.tensor_scalar(out=yc[:], in0=y[:],
                                    scalar1=mv[:, 0:1], scalar2=rs[:],
                                    op0=OP.subtract, op1=OP.mult)
            yg = wk_d.tile([P, F], f32, tag="yg")
            nc.vector.tensor_tensor(out=yg[:], in0=yc[:], in1=g_bc[:],
                                    op=OP.mult)
            yo = wk_d.tile([P, F], f32, tag="yo")
            nc.vector.tensor_tensor(out=yo[:], in0=yg[:], in1=b_bc[:],
                                    op=OP.add)
            nc.sync.dma_start(out=out_d[qc * P:(qc + 1) * P, :], in_=yo[:])

    nc.finalize()
    return nc


def _shard_inputs(inputs):
    import ml_dtypes
    bf = ml_dtypes.bfloat16

    x = np.asarray(inputs["x"], np.float32)
    ei = np.asarray(inputs["edge_index"], np.int64)
    Wq = np.asarray(inputs["Wq"], np.float32)
    bq = np.asarray(inputs["bq"], np.float32)
    Wk = np.asarray(inputs["Wk"], np.float32)
    bk = np.asarray(inputs["bk"], np.float32)
    Wv = np.asarray(inputs["Wv"], np.float32)
    bv = np.asarray(inputs["bv"], np.float32)
    ln_g = np.asarray(inputs["ln_g"], np.float32)
    ln_b = np.asarray(inputs["ln_b"], np.float32)

    adj = np.zeros((N, N), np.bool_)
    adj[ei[0], ei[1]] = True

    x0 = x[0]                                     # [N, F]
    xT = np.ascontiguousarray(x0.T)               # [F, N]

    def b16(a):
        return np.ascontiguousarray(a).astype(bf)

    xT_b = b16(xT).reshape(2, P, N)
    w = {nm: b16(W.T).reshape(2, P, F)
         for nm, W in (("q", Wq), ("k", Wk), ("v", Wv))}
    shared = {
        "xT": xT_b,
        "wqT": w["q"], "wkT": w["k"], "wvT": w["v"],
        "bq8": (bq / np.sqrt(HD)).astype(np.float32).reshape(2, P, 1),
        "bkb": bk.astype(np.float32).reshape(2, P, 1),
        "bvb": bv.reshape(1, F).astype(np.float32),
        "lng": ln_g.reshape(1, F).astype(np.float32),
        "lnb": ln_b.reshape(1, F).astype(np.float32),
    }
    in_maps = []
    for c in range(NCORES):
        rows = slice(c * NQ, (c + 1) * NQ)
        a = adj[rows]                             # [NQ, N]
        m = {
            "xqT": b16(xT[:, rows]).reshape(2, P, NQ),
            "xr": np.ascontiguousarray(x0[rows]).astype(np.float32),
            "madd": np.where(a, np.float32(0.0), np.float32(NEG)
                             ).astype(bf).reshape(4, P, N),
            "mmulT": np.ascontiguousarray(a.T).astype(bf).reshape(32, P, NQ),
        }
        m.update(shared)
        in_maps.append(m)
    return in_maps


def kernel(**inputs):
    global _BUILT, LAST_EXEC_NS, LAST_RESULTS
    from concourse.bass_utils import run_bass_kernel_spmd

    if _BUILT is None:
        _BUILT = _build()
    nc = _BUILT

    in_maps = _shard_inputs(inputs)
    res = run_bass_kernel_spmd(nc, in_maps, core_ids=list(range(NCORES)),
                               trace=TRACE)
    LAST_EXEC_NS = res.exec_time_ns
    LAST_RESULTS = res

    attn_full = np.empty((1, H, N, N), np.float32)
    out_full = np.empty((1, N, F), np.float32)
    for c in range(NCORES):
        rows = slice(c * NQ, (c + 1) * NQ)
        attn_full[0, :, rows, :] = np.asarray(
            res.results[c]["attn"]).astype(np.float32)
        out_full[0, rows, :] = np.asarray(
            res.results[c]["outp"]).astype(np.float32)
    return out_full, attn_full


# revision 9
# speedup vs baseline: 1.0759x; 1.0759x over previous
"""nn_AdaptiveGraphLayer Trainium2 kernel (8 NeuronCores, SPMD).

Sharding: each core owns N/8 = 512 query rows for all H=4 heads.
 - x (node features) replicated -> every core computes K/V for all nodes.
 - adjacency mask built on host, sharded by query rows (multiplicative 0/1,
   [512,4096] for the qk layout and [4096,512] transposed for the kT layout).
 - outputs (attn shard [4,512,4096] bf16, out shard [512,256] f32) are
   gathered on host, so no device collectives are needed.

Device pipeline per core:
  proj:  QT/KT (bf16, head-dim on partitions) and V (natural layout) via PE;
         PSUM evacuated with bias-add on DVE.
  A:     scores = Q^T.T @ K^T per (head, q-chunk) -> PSUM; ACT exp straight
         from PSUM (exp is safe unmasked: |scores| < ~4); DVE fused
         mask-multiply + row-sum (scalar_tensor_tensor accum_out);
         reciprocal; normalize on DVE; DMA attn shard out (bf16).
  B:     scores^T per (head-pair, k-chunk); ACT exp from PSUM; mask-multiply
         split DVE/GPSIMD; PE-accumulate U = V_h^T @ expT, two heads packed
         into one PSUM bank (partition-split).
  C:     PE-transpose U_h, scale by 1/rowsum.
  D:     residual + LayerNorm (bn_stats/bn_aggr), affine, DMA out rows.
"""

import numpy as np

B, N, F, H, HD = 1, 4096, 256, 4, 64
NCORES = 8
NQ = N // NCORES          # 512 query rows per core
P = 128
LN_EPS = 1e-5
GPS_KC_MOD = 4            # kc % GPS_KC_MOD == 0 -> mask-mult on GPSIMD (B)

TRACE = False             # set True (with ntff shim installed) to profile
LAST_EXEC_NS = None
LAST_RESULTS = None

_BUILT = None


def _build():
    from contextlib import ExitStack

    import concourse.bass as bass
    import concourse.bacc as bacc
    import concourse.mybir as mybir
    from concourse.tile import TileContext
    from concourse.masks import make_identity

    dt = mybir.dt
    f32, bf16 = dt.float32, dt.bfloat16
    AF = mybir.ActivationFunctionType
    OP = mybir.AluOpType

    nc = bacc.Bacc("TRN2", target_bir_lowering=False, debug=False,
                   num_devices=NCORES)

    def din(name, shape, dtype):
        return nc.declare_dram_parameter(name, list(shape), dtype, isOutput=False)

    def dout(name, shape, dtype):
        return nc.declare_dram_parameter(name, list(shape), dtype, isOutput=True)

    xT = din("xT", [2, P, N], bf16)          # x^T [f_in, node], f_in-chunked
    xqT = din("xqT", [2, P, NQ], bf16)       # this core's q columns of x^T
    xr = din("xr", [NQ, F], f32)             # residual rows (f32)
    wqT = din("wqT", [2, P, F], bf16)        # Wq.T [f_in, out], f_in-chunked
    wkT = din("wkT", [2, P, F], bf16)
    wvT = din("wvT", [2, P, F], bf16)
    bq8 = din("bq8", [2, P, 1], f32)         # bq / sqrt(hd)
    bkb = din("bkb", [2, P, 1], f32)
    bvb = din("bvb", [1, F], f32)
    lng = din("lng", [1, F], f32)
    lnb = din("lnb", [1, F], f32)
    m01 = din("m01", [4, P, N], bf16)        # 0/1 mask rows, qc-chunked
    mmulT = din("mmulT", [32, P, NQ], bf16)  # 0/1 mask^T, kc-chunked
    attn_d = dout("attn", [H, NQ, N], bf16)
    out_d = dout("outp", [NQ, F], f32)

    with TileContext(nc) as tc, ExitStack() as ctx:
        const = ctx.enter_context(tc.tile_pool(name="const", bufs=1))
        big = ctx.enter_context(tc.tile_pool(name="big", bufs=1))
        wk_ex = ctx.enter_context(tc.tile_pool(name="wk_ex", bufs=3))
        wk_at = ctx.enter_context(tc.tile_pool(name="wk_at", bufs=3))
        wk_b = ctx.enter_context(tc.tile_pool(name="wk_b", bufs=3))
        wk_d = ctx.enter_context(tc.tile_pool(name="wk_d", bufs=1))
        pmm = ctx.enter_context(tc.tile_pool(name="pmm", bufs=3, space="PSUM"))
        pu = ctx.enter_context(tc.tile_pool(name="pu", bufs=2, space="PSUM"))

        # ---- constants / parameters to SBUF ----
        ident = const.tile([P, HD], f32)
        make_identity(nc, ident[0:HD, :])
        make_identity(nc, ident[HD:P, :])

        xT_sb = big.tile([P, 2, N], bf16)
        xqT_sb = big.tile([P, 2, NQ], bf16)
        KT_sb = big.tile([P, 2, N], bf16)
        QT_sb = big.tile([P, 2, NQ], bf16)
        V_sb = big.tile([P, 32, F], bf16)
        m01_sb = big.tile([P, 4, N], bf16)
        mmulT_sb = big.tile([P, 32, NQ], bf16)
        w_sb = {}
        for nm, t in (("q", wqT), ("k", wkT), ("v", wvT)):
            w_sb[nm] = big.tile([P, 2, F], bf16, tag=f"w{nm}", name=f"w{nm}_sb")
            for kc in range(2):
                nc.sync.dma_start(out=w_sb[nm][:, kc, :], in_=t[kc])
        for kc in range(2):
            nc.sync.dma_start(out=xT_sb[:, kc, :], in_=xT[kc])
            nc.sync.dma_start(out=xqT_sb[:, kc, :], in_=xqT[kc])
        for qc in range(4):
            nc.sync.dma_start(out=m01_sb[:, qc, :], in_=m01[qc])
        for kc in range(32):
            nc.sync.dma_start(out=mmulT_sb[:, kc, :], in_=mmulT[kc])

        bq8_sb = const.tile([P, 2], f32)
        bk_sb = const.tile([P, 2], f32)
        for mc in range(2):
            nc.sync.dma_start(out=bq8_sb[:, mc:mc + 1], in_=bq8[mc])
            nc.sync.dma_start(out=bk_sb[:, mc:mc + 1], in_=bkb[mc])

        def bcast(dram_ap):
            # replicate a [1, F] dram row across 128 partitions
            return bass.AP(tensor=dram_ap.tensor, offset=dram_ap.offset,
                           ap=[[0, P]] + list(dram_ap.ap[1:]))

        bv_bc = const.tile([P, F], f32)
        g_bc = const.tile([P, F], f32)
        b_bc = const.tile([P, F], f32)
        nc.sync.dma_start(out=bv_bc[:], in_=bcast(bvb[:]))
        nc.sync.dma_start(out=g_bc[:], in_=bcast(lng[:]))
        nc.sync.dma_start(out=b_bc[:], in_=bcast(lnb[:]))

        xr_sb = const.tile([P, 4, F], f32)
        for qc in range(4):
            nc.sync.dma_start(out=xr_sb[:, qc, :], in_=xr[qc * P:(qc + 1) * P, :])

        eps_sb = const.tile([P, 1], f32)
        nc.vector.memset(eps_sb[:], LN_EPS)
        sums_sb = const.tile([P, H * 4], f32)
        rinv_sb = const.tile([P, H * 4], f32)
        out_sb = const.tile([P, 4, F], f32)

        # ---- projections ----
        for mc in range(2):
            ps = pmm.tile([P, 1024], f32, tag="mm")
            for kc in range(2):
                nc.tensor.matmul(ps[:, 0:NQ],
                                 w_sb["q"][:, kc, mc * P:(mc + 1) * P],
                                 xqT_sb[:, kc, :],
                                 start=(kc == 0), stop=(kc == 1))
            nc.vector.tensor_scalar(out=QT_sb[:, mc, :], in0=ps[:, 0:NQ],
                                    scalar1=1.0 / np.sqrt(HD),
                                    scalar2=bq8_sb[:, mc:mc + 1],
                                    op0=OP.mult, op1=OP.add)
        for mc in range(2):
            for n2 in range(4):      # 4 chunks of 1024
                ps = pmm.tile([P, 1024], f32, tag="mm")
                for j in range(2):
                    n0 = n2 * 1024 + j * 512
                    for kc in range(2):
                        nc.tensor.matmul(ps[:, j * 512:(j + 1) * 512],
                                         w_sb["k"][:, kc, mc * P:(mc + 1) * P],
                                         xT_sb[:, kc, n0:n0 + 512],
                                         start=(kc == 0), stop=(kc == 1))
                nc.vector.tensor_scalar(
                    out=KT_sb[:, mc, n2 * 1024:(n2 + 1) * 1024], in0=ps[:],
                    scalar1=bk_sb[:, mc:mc + 1], scalar2=None, op0=OP.add)
        for nc32 in range(32):
            ps = pmm.tile([P, 1024], f32, tag="mm")
            for kc in range(2):
                nc.tensor.matmul(ps[:, 0:F],
                                 xT_sb[:, kc, nc32 * P:(nc32 + 1) * P],
                                 w_sb["v"][:, kc, :],
                                 start=(kc == 0), stop=(kc == 1))
            nc.vector.scalar_tensor_tensor(out=V_sb[:, nc32, :], in0=ps[:, 0:F],
                                           scalar=0.0, in1=bv_bc[:],
                                           op0=OP.bypass, op1=OP.add)

        def qk_slice(t, h, lo, sz):
            return t[(h % 2) * HD:(h % 2 + 1) * HD, h // 2, lo:lo + sz]

        def phase_a(h, qc):
            idx = h * 4 + qc
            ex = wk_ex.tile([P, N], bf16, tag="ex", name="ex")
            for t4 in range(4):          # 4 psum tiles of 1024 keys
                ps = pmm.tile([P, 1024], f32, tag="mm", name="ps_a")
                for j in range(2):
                    k0 = t4 * 1024 + j * 512
                    nc.tensor.matmul(ps[:, j * 512:(j + 1) * 512],
                                     qk_slice(QT_sb, h, qc * P, P),
                                     qk_slice(KT_sb, h, k0, 512),
                                     start=True, stop=True)
                nc.scalar.activation(out=ex[:, t4 * 1024:(t4 + 1) * 1024],
                                     in_=ps[:], func=AF.Exp)
            at = wk_at.tile([P, N], bf16, tag="at", name="at")
            nc.vector.scalar_tensor_tensor(
                out=at[:], in0=ex[:], scalar=0.0, in1=m01_sb[:, qc, :],
                op0=OP.bypass, op1=OP.mult,
                accum_out=sums_sb[:, idx:idx + 1])
            nc.vector.reciprocal(out=rinv_sb[:, idx:idx + 1],
                                 in_=sums_sb[:, idx:idx + 1])
            nc.vector.tensor_scalar_mul(at[:], at[:], rinv_sb[:, idx:idx + 1])
            nc.sync.dma_start(out=attn_d[h, qc * P:(qc + 1) * P, :], in_=at[:])

        def phase_bc(pair):
            h0, h1 = 2 * pair, 2 * pair + 1
            # U for both heads packed into one PSUM bank: h0 on partitions
            # 0:64, h1 on 64:128
            up = pu.tile([P, NQ], f32, tag="u", name="up")
            for kc in range(32):
                ps = pmm.tile([P, 1024], f32, tag="mm", name="ps_b")
                for j, h in enumerate((h0, h1)):
                    nc.tensor.matmul(ps[:, j * NQ:(j + 1) * NQ],
                                     qk_slice(KT_sb, h, kc * P, P),
                                     qk_slice(QT_sb, h, 0, NQ),
                                     start=True, stop=True)
                et = wk_b.tile([P, 1024], bf16, tag="et", name="et")
                nc.scalar.activation(out=et[:], in_=ps[:], func=AF.Exp)
                em = wk_b.tile([P, 1024], bf16, tag="em", name="em")
                eng = nc.gpsimd if (kc % GPS_KC_MOD == 0) else nc.vector
                for j in range(2):
                    eng.tensor_tensor(out=em[:, j * NQ:(j + 1) * NQ],
                                      in0=et[:, j * NQ:(j + 1) * NQ],
                                      in1=mmulT_sb[:, kc, :], op=OP.mult)
                for j, h in enumerate((h0, h1)):
                    nc.tensor.matmul(up[j * HD:(j + 1) * HD, :],
                                     V_sb[:, kc, h * HD:(h + 1) * HD],
                                     em[:, j * NQ:(j + 1) * NQ],
                                     start=(kc == 0), stop=(kc == 31))
            us = wk_b.tile([P, NQ], f32, tag="us", name="us")
            nc.vector.tensor_copy(us[:], up[:])
            for j, h in enumerate((h0, h1)):
                tp = pu.tile([P, 4, HD], f32, tag="u", name="tp")
                for qc in range(4):
                    nc.tensor.transpose(
                        tp[:, qc, :],
                        us[j * HD:(j + 1) * HD, qc * P:(qc + 1) * P],
                        ident[j * HD:(j + 1) * HD, :])
                for qc in range(4):
                    nc.vector.tensor_scalar_mul(
                        out_sb[:, qc, h * HD:(h + 1) * HD], tp[:, qc, :],
                        rinv_sb[:, h * 4 + qc:h * 4 + qc + 1])

        # interleave A and B/C for engine overlap
        phase_a(0, 0); phase_a(0, 1); phase_a(0, 2); phase_a(0, 3)
        phase_a(1, 0); phase_a(1, 1); phase_a(1, 2); phase_a(1, 3)
        phase_bc(0)
        phase_a(2, 0); phase_a(2, 1); phase_a(2, 2); phase_a(2, 3)
        phase_a(3, 0); phase_a(3, 1); phase_a(3, 2); phase_a(3, 3)
        phase_bc(1)

        # ---- phase D: residual + layernorm + affine ----
        for qc in range(4):
            y = wk_d.tile([P, F], f32, tag="y", name="y")
            nc.vector.tensor_tensor(out=y[:], in0=out_sb[:, qc, :],
                                    in1=xr_sb[:, qc, :], op=OP.add)
            st = wk_d.tile([P, 6], f32, tag="st", name="st")
            nc.vector.bn_stats(out=st[:], in_=y[:])
            mv = wk_d.tile([P, 2], f32, tag="mv", name="mv")
            nc.vector.bn_aggr(out=mv[:], in_=st[:])
            sd = wk_d.tile([P, 1], f32, tag="sd", name="sd")
            nc.scalar.activation(out=sd[:], in_=mv[:, 1:2], func=AF.Sqrt,
                                 bias=eps_sb[:], scale=1.0)
            rs = wk_d.tile([P, 1], f32, tag="rs", name="rs")
            nc.vector.reciprocal(out=rs[:], in_=sd[:])
            yc = wk_d.tile([P, F], f32, tag="yc", name="yc")
            nc.vector.tensor_scalar(out=yc[:], in0=y[:],
                                    scalar1=mv[:, 0:1], scalar2=rs[:],
                                    op0=OP.subtract, op1=OP.mult)
            yg = wk_d.tile([P, F], f32, tag="yg", name="yg")
            nc.vector.tensor_tensor(out=yg[:], in0=yc[:], in1=g_bc[:],
                                    op=OP.mult)
            yo = wk_d.tile([P, F], f32, tag="yo", name="yo")
            nc.vector.tensor_tensor(out=yo[:], in0=yg[:], in1=b_bc[:],
                                    op=OP.add)
            nc.sync.dma_start(out=out_d[qc * P:(qc + 1) * P, :], in_=yo[:])

    nc.finalize()
    return nc


def _shard_inputs(inputs):
    import ml_dtypes
    bf = ml_dtypes.bfloat16

    x = np.asarray(inputs["x"], np.float32)
    ei = np.asarray(inputs["edge_index"], np.int64)
    Wq = np.asarray(inputs["Wq"], np.float32)
    bq = np.asarray(inputs["bq"], np.float32)
    Wk = np.asarray(inputs["Wk"], np.float32)
    bk = np.asarray(inputs["bk"], np.float32)
    Wv = np.asarray(inputs["Wv"], np.float32)
    bv = np.asarray(inputs["bv"], np.float32)
    ln_g = np.asarray(inputs["ln_g"], np.float32)
    ln_b = np.asarray(inputs["ln_b"], np.float32)

    adj = np.zeros((N, N), np.bool_)
    adj[ei[0], ei[1]] = True

    x0 = x[0]                                     # [N, F]
    xT = np.ascontiguousarray(x0.T)               # [F, N]

    def b16(a):
        return np.ascontiguousarray(a).astype(bf)

    xT_b = b16(xT).reshape(2, P, N)
    w = {nm: b16(W.T).reshape(2, P, F)
         for nm, W in (("q", Wq), ("k", Wk), ("v", Wv))}
    shared = {
        "xT": xT_b,
        "wqT": w["q"], "wkT": w["k"], "wvT": w["v"],
        "bq8": (bq / np.sqrt(HD)).astype(np.float32).reshape(2, P, 1),
        "bkb": bk.astype(np.float32).reshape(2, P, 1),
        "bvb": bv.reshape(1, F).astype(np.float32),
        "lng": ln_g.reshape(1, F).astype(np.float32),
        "lnb": ln_b.reshape(1, F).astype(np.float32),
    }
    in_maps = []
    for c in range(NCORES):
        rows = slice(c * NQ, (c + 1) * NQ)
        a = adj[rows]                             # [NQ, N]
        m = {
            "xqT": b16(xT[:, rows]).reshape(2, P, NQ),
            "xr": np.ascontiguousarray(x0[rows]).astype(np.float32),
            "m01": a.astype(bf).reshape(4, P, N),
            "mmulT": np.ascontiguousarray(a.T).astype(bf).reshape(32, P, NQ),
        }
        m.update(shared)
        in_maps.append(m)
    return in_maps


def kernel(**inputs):
    global _BUILT, LAST_EXEC_NS, LAST_RESULTS
    from concourse.bass_utils import run_bass_kernel_spmd

    if _BUILT is None:
        _BUILT = _build()
    nc = _BUILT

    in_maps = _shard_inputs(inputs)
    res = run_bass_kernel_spmd(nc, in_maps, core_ids=list(range(NCORES)),
                               trace=TRACE)
    LAST_EXEC_NS = res.exec_time_ns
    LAST_RESULTS = res

    attn_full = np.empty((1, H, N, N), np.float32)
    out_full = np.empty((1, N, F), np.float32)
    for c in range(NCORES):
        rows = slice(c * NQ, (c + 1) * NQ)
        attn_full[0, :, rows, :] = np.asarray(
            res.results[c]["attn"]).astype(np.float32)
        out_full[0, rows, :] = np.asarray(
            res.results[c]["outp"]).astype(np.float32)
    return out_full, attn_full


# revision 10
# speedup vs baseline: 1.1350x; 1.0549x over previous
"""nn_AdaptiveGraphLayer Trainium2 kernel (8 NeuronCores, SPMD).

Sharding: each core owns N/8 = 512 query rows for all H=4 heads.
 - x (node features) replicated -> every core computes K/V for all nodes.
 - adjacency mask built on host, sharded by query rows (multiplicative 0/1,
   [512,4096] qk layout and [4096,512] transposed kT layout).
 - outputs (attn shard [4,512,4096] bf16, out shard [512,256] f32) gathered
   on host; no device collectives.

Per-head device pipeline (B phase runs FIRST: it produces the softmax row
sums for free):
  B:   scores^T chunks via PE; ACT exp straight from PSUM (safe unmasked:
       |scores| < ~4); DVE mask-multiply (2x bf16); PE-accumulate
       U~ = [V_h | 1]^T @ expT  -> row 64 of U~ is the masked-exp row sum.
  C:   evacuate U~, PE-transpose, reciprocal of the sums column -> rinv;
       scale the V-aggregation by rinv.
  A:   scores chunks via PE; ACT exp from PSUM; DVE mask-multiply;
       DVE normalize with rinv (4x bf16); DMA attn shard out.
  D:   residual + LayerNorm (bn_stats/bn_aggr), affine, DMA out rows.
"""

import numpy as np

B, N, F, H, HD = 1, 4096, 256, 4, 64
NCORES = 8
NQ = N // NCORES          # 512 query rows per core
P = 128
HD1 = HD + 1              # V plus the ones column
LN_EPS = 1e-5

TRACE = False             # set True (with ntff shim installed) to profile
LAST_EXEC_NS = None
LAST_RESULTS = None

_BUILT = None


def _build():
    from contextlib import ExitStack

    import concourse.bass as bass
    import concourse.bacc as bacc
    import concourse.mybir as mybir
    from concourse.tile import TileContext
    from concourse.masks import make_identity

    dt = mybir.dt
    f32, bf16 = dt.float32, dt.bfloat16
    AF = mybir.ActivationFunctionType
    OP = mybir.AluOpType

    nc = bacc.Bacc("TRN2", target_bir_lowering=False, debug=False,
                   num_devices=NCORES)

    def din(name, shape, dtype):
        return nc.declare_dram_parameter(name, list(shape), dtype, isOutput=False)

    def dout(name, shape, dtype):
        return nc.declare_dram_parameter(name, list(shape), dtype, isOutput=True)

    xT = din("xT", [2, P, N], bf16)          # x^T [f_in, node], f_in-chunked
    xqT = din("xqT", [2, P, NQ], bf16)       # this core's q columns of x^T
    xr = din("xr", [NQ, F], f32)             # residual rows (f32)
    wqT = din("wqT", [2, P, F], bf16)        # Wq.T [f_in, out], f_in-chunked
    wkT = din("wkT", [2, P, F], bf16)
    wvT = din("wvT", [2, P, F], bf16)
    bq8 = din("bq8", [2, P, 1], f32)         # bq / sqrt(hd)
    bkb = din("bkb", [2, P, 1], f32)
    bvb = din("bvb", [1, F], f32)
    lng = din("lng", [1, F], f32)
    lnb = din("lnb", [1, F], f32)
    m01 = din("m01", [4, P, N], bf16)        # 0/1 mask rows, qc-chunked
    mmulT = din("mmulT", [32, P, NQ], bf16)  # 0/1 mask^T, kc-chunked
    attn_d = dout("attn", [H, NQ, N], bf16)
    out_d = dout("outp", [NQ, F], f32)

    with TileContext(nc) as tc, ExitStack() as ctx:
        const = ctx.enter_context(tc.tile_pool(name="const", bufs=1))
        big = ctx.enter_context(tc.tile_pool(name="big", bufs=1))
        wk_ex = ctx.enter_context(tc.tile_pool(name="wk_ex", bufs=3))
        wk_at = ctx.enter_context(tc.tile_pool(name="wk_at", bufs=3))
        wk_b = ctx.enter_context(tc.tile_pool(name="wk_b", bufs=3))
        wk_d = ctx.enter_context(tc.tile_pool(name="wk_d", bufs=1))
        pmm = ctx.enter_context(tc.tile_pool(name="pmm", bufs=3, space="PSUM"))
        pu = ctx.enter_context(tc.tile_pool(name="pu", bufs=2, space="PSUM"))

        # ---- constants / parameters to SBUF ----
        ident = const.tile([HD1, HD1], f32)
        make_identity(nc, ident[:])

        xT_sb = big.tile([P, 2, N], bf16)
        xqT_sb = big.tile([P, 2, NQ], bf16)
        KT_sb = big.tile([P, 2, N], bf16)
        QT_sb = big.tile([P, 2, NQ], bf16)
        V_sb = big.tile([P, 32, H, HD1], bf16)     # V plus ones column
        m01_sb = big.tile([P, 4, N], bf16)
        mmulT_sb = big.tile([P, 32, NQ], bf16)
        nc.vector.memset(V_sb[:, :, :, HD:HD1], 1.0)
        w_sb = {}
        for nm, t in (("q", wqT), ("k", wkT), ("v", wvT)):
            w_sb[nm] = big.tile([P, 2, F], bf16, tag=f"w{nm}", name=f"w{nm}_sb")
            for kc in range(2):
                nc.sync.dma_start(out=w_sb[nm][:, kc, :], in_=t[kc])
        for kc in range(2):
            nc.sync.dma_start(out=xT_sb[:, kc, :], in_=xT[kc])
            nc.sync.dma_start(out=xqT_sb[:, kc, :], in_=xqT[kc])
        for qc in range(4):
            nc.sync.dma_start(out=m01_sb[:, qc, :], in_=m01[qc])
        for kc in range(32):
            nc.sync.dma_start(out=mmulT_sb[:, kc, :], in_=mmulT[kc])

        bq8_sb = const.tile([P, 2], f32)
        bk_sb = const.tile([P, 2], f32)
        for mc in range(2):
            nc.sync.dma_start(out=bq8_sb[:, mc:mc + 1], in_=bq8[mc])
            nc.sync.dma_start(out=bk_sb[:, mc:mc + 1], in_=bkb[mc])

        def bcast(dram_ap):
            # replicate a [1, F] dram row across 128 partitions
            return bass.AP(tensor=dram_ap.tensor, offset=dram_ap.offset,
                           ap=[[0, P]] + list(dram_ap.ap[1:]))

        bv_bc = const.tile([P, F], f32)
        g_bc = const.tile([P, F], f32)
        b_bc = const.tile([P, F], f32)
        nc.sync.dma_start(out=bv_bc[:], in_=bcast(bvb[:]))
        nc.sync.dma_start(out=g_bc[:], in_=bcast(lng[:]))
        nc.sync.dma_start(out=b_bc[:], in_=bcast(lnb[:]))

        xr_sb = const.tile([P, 4, F], f32)
        for qc in range(4):
            nc.sync.dma_start(out=xr_sb[:, qc, :], in_=xr[qc * P:(qc + 1) * P, :])

        eps_sb = const.tile([P, 1], f32)
        nc.vector.memset(eps_sb[:], LN_EPS)
        rinv_sb = const.tile([P, H * 4], f32)
        out_sb = const.tile([P, 4, F], f32)

        # ---- projections ----
        for mc in range(2):
            ps = pmm.tile([P, 1024], f32, tag="mm", name="ps_q")
            for kc in range(2):
                nc.tensor.matmul(ps[:, 0:NQ],
                                 w_sb["q"][:, kc, mc * P:(mc + 1) * P],
                                 xqT_sb[:, kc, :],
                                 start=(kc == 0), stop=(kc == 1))
            nc.vector.tensor_scalar(out=QT_sb[:, mc, :], in0=ps[:, 0:NQ],
                                    scalar1=1.0 / np.sqrt(HD),
                                    scalar2=bq8_sb[:, mc:mc + 1],
                                    op0=OP.mult, op1=OP.add)
        for mc in range(2):
            for n2 in range(4):      # 4 chunks of 1024
                ps = pmm.tile([P, 1024], f32, tag="mm", name="ps_k")
                for j in range(2):
                    n0 = n2 * 1024 + j * 512
                    for kc in range(2):
                        nc.tensor.matmul(ps[:, j * 512:(j + 1) * 512],
                                         w_sb["k"][:, kc, mc * P:(mc + 1) * P],
                                         xT_sb[:, kc, n0:n0 + 512],
                                         start=(kc == 0), stop=(kc == 1))
                nc.scalar.activation(
                    out=KT_sb[:, mc, n2 * 1024:(n2 + 1) * 1024], in_=ps[:],
                    func=AF.Identity, bias=bk_sb[:, mc:mc + 1], scale=1.0)
        for nc32 in range(32):
            ps = pmm.tile([P, 1024], f32, tag="mm", name="ps_v")
            for kc in range(2):
                nc.tensor.matmul(ps[:, 0:F],
                                 xT_sb[:, kc, nc32 * P:(nc32 + 1) * P],
                                 w_sb["v"][:, kc, :],
                                 start=(kc == 0), stop=(kc == 1))
            nc.vector.scalar_tensor_tensor(
                out=V_sb[:, nc32, :, 0:HD],
                in0=ps[:, 0:F].rearrange("p (h d) -> p h d", h=H),
                scalar=0.0, in1=bv_bc[:].rearrange("p (h d) -> p h d", h=H),
                op0=OP.bypass, op1=OP.add)

        def qk_slice(t, h, lo, sz):
            return t[(h % 2) * HD:(h % 2 + 1) * HD, h // 2, lo:lo + sz]

        def phase_b(h):
            # U~ = [V_h | 1]^T @ (exp(scores^T) * mask^T), accumulated in PSUM
            up = pu.tile([HD1, NQ], f32, tag="u", name="up")
            for t16 in range(16):            # kc pairs
                kc0 = 2 * t16
                ps = pmm.tile([P, 1024], f32, tag="mm", name="ps_b")
                for j in range(2):
                    nc.tensor.matmul(ps[:, j * NQ:(j + 1) * NQ],
                                     qk_slice(KT_sb, h, (kc0 + j) * P, P),
                                     qk_slice(QT_sb, h, 0, NQ),
                                     start=True, stop=True)
                et = wk_b.tile([P, 1024], bf16, tag="et", name="et")
                nc.scalar.activation(out=et[:], in_=ps[:], func=AF.Exp)
                em = wk_b.tile([P, 1024], bf16, tag="em", name="em")
                nc.vector.tensor_tensor(
                    out=em[:].rearrange("p (j q) -> p j q", j=2),
                    in0=et[:].rearrange("p (j q) -> p j q", j=2),
                    in1=mmulT_sb[:, kc0:kc0 + 2, :], op=OP.mult)
                for j in range(2):
                    nc.tensor.matmul(up[:],
                                     V_sb[:, kc0 + j, h, :],
                                     em[:, j * NQ:(j + 1) * NQ],
                                     start=(t16 == 0 and j == 0),
                                     stop=(t16 == 15 and j == 1))
            # C: evacuate, transpose, rinv, scale
            us = wk_b.tile([HD1, NQ], f32, tag="us", name="us")
            nc.vector.tensor_copy(us[:], up[:])
            tp = pu.tile([P, 4, HD1], f32, tag="u", name="tp")
            for qc in range(4):
                nc.tensor.transpose(tp[:, qc, :],
                                    us[:, qc * P:(qc + 1) * P], ident[:])
            for qc in range(4):
                idx = h * 4 + qc
                nc.vector.reciprocal(out=rinv_sb[:, idx:idx + 1],
                                     in_=tp[:, qc, HD:HD1])
                nc.vector.tensor_scalar_mul(
                    out_sb[:, qc, h * HD:(h + 1) * HD], tp[:, qc, 0:HD],
                    rinv_sb[:, idx:idx + 1])

        def phase_a(h, qc):
            idx = h * 4 + qc
            ex = wk_ex.tile([P, N], bf16, tag="ex", name="ex")
            for t4 in range(4):          # 4 psum tiles of 1024 keys
                ps = pmm.tile([P, 1024], f32, tag="mm", name="ps_a")
                for j in range(2):
                    k0 = t4 * 1024 + j * 512
                    nc.tensor.matmul(ps[:, j * 512:(j + 1) * 512],
                                     qk_slice(QT_sb, h, qc * P, P),
                                     qk_slice(KT_sb, h, k0, 512),
                                     start=True, stop=True)
                nc.scalar.activation(out=ex[:, t4 * 1024:(t4 + 1) * 1024],
                                     in_=ps[:], func=AF.Exp)
            at = wk_at.tile([P, N], bf16, tag="at", name="at")
            nc.vector.tensor_tensor(out=at[:], in0=ex[:],
                                    in1=m01_sb[:, qc, :], op=OP.mult)
            nc.vector.tensor_scalar_mul(at[:], at[:], rinv_sb[:, idx:idx + 1])
            nc.sync.dma_start(out=attn_d[h, qc * P:(qc + 1) * P, :], in_=at[:])

        for h in range(H):
            phase_b(h)
            for qc in range(4):
                phase_a(h, qc)

        # ---- phase D: residual + layernorm + affine ----
        for qc in range(4):
            y = wk_d.tile([P, F], f32, tag="y", name="y")
            nc.vector.tensor_tensor(out=y[:], in0=out_sb[:, qc, :],
                                    in1=xr_sb[:, qc, :], op=OP.add)
            st = wk_d.tile([P, 6], f32, tag="st", name="st")
            nc.vector.bn_stats(out=st[:], in_=y[:])
            mv = wk_d.tile([P, 2], f32, tag="mv", name="mv")
            nc.vector.bn_aggr(out=mv[:], in_=st[:])
            sd = wk_d.tile([P, 1], f32, tag="sd", name="sd")
            nc.scalar.activation(out=sd[:], in_=mv[:, 1:2], func=AF.Sqrt,
                                 bias=eps_sb[:], scale=1.0)
            rs = wk_d.tile([P, 1], f32, tag="rs", name="rs")
            nc.vector.reciprocal(out=rs[:], in_=sd[:])
            yc = wk_d.tile([P, F], f32, tag="yc", name="yc")
            nc.vector.tensor_scalar(out=yc[:], in0=y[:],
                                    scalar1=mv[:, 0:1], scalar2=rs[:],
                                    op0=OP.subtract, op1=OP.mult)
            yg = wk_d.tile([P, F], f32, tag="yg", name="yg")
            nc.vector.tensor_tensor(out=yg[:], in0=yc[:], in1=g_bc[:],
                                    op=OP.mult)
            yo = wk_d.tile([P, F], f32, tag="yo", name="yo")
            nc.vector.tensor_tensor(out=yo[:], in0=yg[:], in1=b_bc[:],
                                    op=OP.add)
            nc.sync.dma_start(out=out_d[qc * P:(qc + 1) * P, :], in_=yo[:])

    nc.finalize()
    return nc


def _shard_inputs(inputs):
    import ml_dtypes
    bf = ml_dtypes.bfloat16

    x = np.asarray(inputs["x"], np.float32)
    ei = np.asarray(inputs["edge_index"], np.int64)
    Wq = np.asarray(inputs["Wq"], np.float32)
    bq = np.asarray(inputs["bq"], np.float32)
    Wk = np.asarray(inputs["Wk"], np.float32)
    bk = np.asarray(inputs["bk"], np.float32)
    Wv = np.asarray(inputs["Wv"], np.float32)
    bv = np.asarray(inputs["bv"], np.float32)
    ln_g = np.asarray(inputs["ln_g"], np.float32)
    ln_b = np.asarray(inputs["ln_b"], np.float32)

    adj = np.zeros((N, N), np.bool_)
    adj[ei[0], ei[1]] = True

    x0 = x[0]                                     # [N, F]
    xT = np.ascontiguousarray(x0.T)               # [F, N]

    def b16(a):
        return np.ascontiguousarray(a).astype(bf)

    xT_b = b16(xT).reshape(2, P, N)
    w = {nm: b16(W.T).reshape(2, P, F)
         for nm, W in (("q", Wq), ("k", Wk), ("v", Wv))}
    shared = {
        "xT": xT_b,
        "wqT": w["q"], "wkT": w["k"], "wvT": w["v"],
        "bq8": (bq / np.sqrt(HD)).astype(np.float32).reshape(2, P, 1),
        "bkb": bk.astype(np.float32).reshape(2, P, 1),
        "bvb": bv.reshape(1, F).astype(np.float32),
        "lng": ln_g.reshape(1, F).astype(np.float32),
        "lnb": ln_b.reshape(1, F).astype(np.float32),
    }
    in_maps = []
    for c in range(NCORES):
        rows = slice(c * NQ, (c + 1) * NQ)
        a = adj[rows]                             # [NQ, N]
        m = {
            "xqT": b16(xT[:, rows]).reshape(2, P, NQ),
            "xr": np.ascontiguousarray(x0[rows]).astype(np.float32),
            "m01": a.astype(bf).reshape(4, P, N),
            "mmulT": np.ascontiguousarray(a.T).astype(bf).reshape(32, P, NQ),
        }
        m.update(shared)
        in_maps.append(m)
    return in_maps


def kernel(**inputs):
    global _BUILT, LAST_EXEC_NS, LAST_RESULTS
    from concourse.bass_utils import run_bass_kernel_spmd

    if _BUILT is None:
        _BUILT = _build()
    nc = _BUILT

    in_maps = _shard_inputs(inputs)
    res = run_bass_kernel_spmd(nc, in_maps, core_ids=list(range(NCORES)),
                               trace=TRACE)
    LAST_EXEC_NS = res.exec_time_ns
    LAST_RESULTS = res

    attn_full = np.empty((1, H, N, N), np.float32)
    out_full = np.empty((1, N, F), np.float32)
    for c in range(NCORES):
        rows = slice(c * NQ, (c + 1) * NQ)
        attn_full[0, :, rows, :] = np.asarray(
            res.results[c]["attn"]).astype(np.float32)
        out_full[0, rows, :] = np.asarray(
            res.results[c]["outp"]).astype(np.float32)
    return out_full, attn_full
